# revision 25
# baseline (speedup 1.0000x reference)
"""DLoRF low-rank linear kernel for Trainium2 (8 NeuronCores, SPMD).

Computes  out = x @ U @ diag(s * mask) @ V.T  for
  x [8, 2048, 4096] f32, U [4096, 512], V [4096, 512], s/mask [512].

Strategy: data-parallel over the batch dim (one batch element per core).
Host folds diag(s*mask) into U, pre-transposes x per batch element
(feature-major) and converts everything to bf16 -- so the device does
no transposes at all and every matmul streams at 1 column/cycle with
fast (FWL) weight loads. Per core:

  GEMM1: t.T[k, tok] += U_s[f, k].T @ xT[f, tok]   (psum f32, evict bf16)
  GEMM2: out[tok, o] += t.T[k, tok].T @ V.T[k, o]  (psum f32, store f32)

Both GEMMs run with 512-wide moving operands (one full PSUM bank), in
512-token chunks with GEMM2 skewed one chunk behind GEMM1.  A short
burst of dummy matmuls at the head keeps the PE busy while the first
DMAs land so the HAM clock gate reaches 2.4 GHz before real work.
bf16 inputs with f32 accumulation give rel-l2 error ~1.5e-3.
"""

import numpy as np
import ml_dtypes

import concourse.bacc as bacc
import concourse.mybir as mybir
import concourse.tile as tile
from concourse.bass import _add_dep_helper
from concourse.bass_utils import run_bass_kernel_spmd

B, S, IN_F, OUT_F, KR = 8, 2048, 4096, 4096, 512
P = 128
N_CORES = 8
KT = IN_F // P  # 32 feature tiles (contraction of GEMM1)
MT = KR // P  # 4 rank tiles (contraction of GEMM2)
TC = 512  # token chunk (moving free dim of GEMM1)
NCH = S // TC  # 4 chunks
OW = 512  # out-feature chunk (moving free dim of GEMM2)
OC = OUT_F // OW  # 8
XG = 8  # x DMA groups per chunk (4 kt-tiles, 512KB each)
NW = 9  # warmup matmuls (~3.8us cold) to lift the HAM clock gate

BF16 = mybir.dt.bfloat16
F32 = mybir.dt.float32


def build():
    nc = bacc.Bacc()
    # x.T per core, pre-arranged: xt[p, kt, tok] = x[tok, kt*128+p]
    xt_d = nc.declare_dram_parameter("xt", [P, KT, S], BF16, isOutput=False)
    # us[p, kt, m, q] = (U*s)[kt*128+p, m*128+q]  (kt-major so U streams
    # in lockstep with the kt-outer GEMM1 loop)
    us_d = nc.declare_dram_parameter("us", [P, KT, MT, P], BF16, isOutput=False)
    # vt[p, m, o] = V[o, m*128+p]
    vt_d = nc.declare_dram_parameter("vt", [P, MT, OUT_F], BF16, isOutput=False)
    out_d = nc.declare_dram_parameter("out", [S, OUT_F], F32, isOutput=True)

    with tile.TileContext(nc) as tc:
        with (
            tc.tile_pool(name="wpool", bufs=1) as wpool,
            tc.tile_pool(name="xtp", bufs=3) as xtp,
            tc.tile_pool(name="ttp", bufs=2) as ttp,
            tc.tile_pool(name="ostage", bufs=4) as ostp,
            tc.tile_pool(name="wrm", bufs=1) as wrmp,
            tc.tile_pool(name="ps1", bufs=1, space="PSUM") as ps1,
            tc.tile_pool(name="ps2", bufs=4, space="PSUM") as ps2,
        ):
            # Warmup: the PE clock gate (HAM) starts at 1.2 GHz and only
            # reaches 2.4 GHz after ~3.4us of sustained activity.  Run
            # dummy matmuls on a memset tile while the first x/U DMAs are
            # in flight so the stream is continuously busy from ~7us on.
            wtile = wrmp.tile([P, 256], BF16)
            nc.vector.memset(wtile[:], 0.0)
            wps = ps2.tile([P, OW], F32, tag="p2")
            for _ in range(NW):
                nc.tensor.matmul(
                    wps[:, 0:256], wtile[:, 0:128], wtile[:], start=True, stop=True
                )

            # Weights resident all kernel on the gpsimd (SWDGE) queue;
            # sync ring carries the x stream, scalar ring the stores.
            # U streams in 8 kt-groups paired with the x kt-groups so
            # chunk 0's kt-outer loop is fed at ~1MB per 3.4us of PE work.
            us_t = wpool.tile([P, KT, MT, P], BF16)
            vt_full = wpool.tile([P, MT, OUT_F], BF16)
            us_dmas = [
                nc.gpsimd.dma_start(
                    us_t[:, g * 4 : (g + 1) * 4],
                    us_d[:, g * 4 : (g + 1) * 4],
                )
                for g in range(8)
            ]
            vt_dmas = [
                nc.gpsimd.dma_start(
                    vt_full[:, :, oc * OW : (oc + 1) * OW],
                    vt_d[:, :, oc * OW : (oc + 1) * OW],
                )
                for oc in range(OC)
            ]

            xg_dmas = {}
            xts = {}

            def load_x(c, ng):
                # chunk 0 loads in fine 2-kt groups (256KB) so the first
                # matmuls unblock ASAP; later chunks use coarse groups
                # (fewer sem waits on the PE queue)
                xt_sb = xtp.tile([P, KT, TC], BF16, tag="xt")
                gk = KT // ng
                for g in range(ng):
                    xg_dmas[(c, g)] = nc.sync.dma_start(
                        xt_sb[:, g * gk : (g + 1) * gk, :],
                        xt_d[:, g * gk : (g + 1) * gk, c * TC : (c + 1) * TC],
                    )
                xts[c] = xt_sb

            def gemm1(c):
                # kt-outer with 4 concurrent psum accumulators: each
                # arriving 512KB x-group + 512KB U-group feeds 16 matmuls
                # (3.4us), so chunk 0 streams gap-free behind the DMA.
                xt_sb = xts.pop(c)
                tt = ttp.tile([P, MT, TC], BF16, tag="tt")
                p1s = [
                    ps1.tile([P, TC], F32, tag=f"p1_{m}", name=f"p1_{m}")
                    for m in range(MT)
                ]
                for kt in range(KT):
                    for m in range(MT):
                        nc.tensor.matmul(
                            p1s[m][:],
                            us_t[:, kt, m, :],
                            xt_sb[:, kt, :],
                            start=(kt == 0),
                            stop=(kt == KT - 1),
                        )
                for m in range(MT):
                    # psum evict + f32->bf16 cast, alternating engines
                    copy_eng = nc.scalar.copy if m % 2 == 0 else nc.vector.tensor_copy
                    copy_eng(tt[:, m, :], p1s[m][:])
                return tt

            def gemm2(c, tt):
                for ts in range(TC // P):
                    tok0 = c * TC + ts * P
                    for pair in range(OC // 2):
                        ost = ostp.tile([P, 2 * OW], F32, tag="ost")
                        for half in range(2):
                            oc = pair * 2 + half
                            p2 = ps2.tile([P, OW], F32, tag="p2")
                            for m in range(MT):
                                nc.tensor.matmul(
                                    p2[:],
                                    tt[:, m, ts * P : (ts + 1) * P],
                                    vt_full[:, m, oc * OW : (oc + 1) * OW],
                                    start=(m == 0),
                                    stop=(m == MT - 1),
                                )
                            copy_eng = (
                                nc.vector.tensor_copy if half == 0 else nc.scalar.copy
                            )
                            copy_eng(ost[:, half * OW : (half + 1) * OW], p2[:])
                        # 1MB-wide staged store halves the scalar-queue
                        # DMA-issue count
                        nc.scalar.dma_start(
                            out_d[tok0 : tok0 + P, pair * 2 * OW : (pair + 1) * 2 * OW],
                            ost[:],
                        )

            load_x(0, 8)
            load_x(1, 8)
            tts = {}
            for c in range(NCH + 1):
                if c < NCH:
                    tts[c] = gemm1(c)
                    if c + 2 < NCH:
                        load_x(c + 2, 8)
                if c >= 1:
                    gemm2(c - 1, tts.pop(c - 1))

            # HBM sequencing: V.T streams only after chunk 1's x is fully
            # requested (the early window goes to x + U_s), and chunks
            # 2/3 of x queue behind V.T (they are not needed until much
            # later, V.T is needed first by GEMM2 chunk 0).
            for vdma in vt_dmas:
                _add_dep_helper(
                    vdma.ins,
                    xg_dmas[(1, 7)].ins,
                    sync=True,
                    reason="stagger vt loads behind early x stream",
                )
            _add_dep_helper(
                xg_dmas[(2, 0)].ins,
                vt_dmas[-1].ins,
                sync=True,
                reason="x chunk 2/3 after vt",
            )
    nc.finalize()
    return nc


_NC_CACHE = {}


def _get_nc():
    key = "main"
    if key not in _NC_CACHE:
        _NC_CACHE[key] = build()
    return _NC_CACHE[key]


def kernel(x, U, V, s, mask, _trace=False, _trace_kwargs=None):
    x = np.asarray(x)
    U = np.asarray(U)
    V = np.asarray(V)
    s = np.asarray(s)
    mask = np.asarray(mask)
    bf16 = ml_dtypes.bfloat16
    s_masked = (s.astype(np.float32) * mask.astype(np.float32)).astype(np.float32)
    U_s = U.astype(np.float32) * s_masked[None, :]
    # pre-arrange weights into the kernel's partition-major SBUF layout
    us_prep = np.ascontiguousarray(
        U_s.reshape(KT, P, MT, P).transpose(1, 0, 2, 3).astype(bf16)
    )  # [P, KT, MT, P]
    vt_prep = np.ascontiguousarray(
        V.astype(np.float32).T.reshape(MT, P, OUT_F).transpose(1, 0, 2).astype(bf16)
    )  # [P, MT, OUT_F]
    nc = _get_nc()
    in_maps = []
    for b in range(B):
        xt_b = np.ascontiguousarray(
            x[b].T.reshape(KT, P, S).transpose(1, 0, 2).astype(bf16)
        )  # [P, KT, S]
        in_maps.append({"xt": xt_b, "us": us_prep, "vt": vt_prep})
    res = run_bass_kernel_spmd(
        nc, in_maps, list(range(N_CORES)), trace=_trace, **(_trace_kwargs or {})
    )
    out = np.stack([res.results[b]["out"] for b in range(B)], axis=0)
    if _trace:
        return out, res
    return out
